# revision 6
# baseline (speedup 1.0000x reference)
"""Cross-attention kernel for Trainium2 (8 NeuronCores).

Problem (reference semantics, all fp32):
    q = split_heads(dec @ q_w + q_b)        # [B,H,Sq,64]
    k = split_heads(enc @ k_w + k_b)        # [B,H,Sk,64]
    v = split_heads(enc @ v_w + v_b)        # [B,H,Sk,64]
    a = softmax(mask(q k^T / 8))
    out = merge_heads(a @ v) @ o_w + o_b    # [B,Sq,1024]
with B=4, Sq=1024, Sk=2048, D=1024, H=16.

Sharding: batch x head-group. Core c handles batch b=c//2 and heads
hg*8..hg*8+8 (hg=c%2), i.e. a 512-wide feature slice of the q/k/v
projections and the matching 512 rows of o_w. Each core emits a full
[1024,1024] partial of its batch's output; the host sums the two
partials per batch and adds o_b.

On-core dataflow (all matmuls in fp32r = full PE rate, ~1e-4 rounding):
  decT/encT via PE transposes -> qT/kT [f,s] projections (bias fused in
  the PSUM->SBUF activation), v [sk,f] with bias folded in as a rank-1
  matmul and a ones column appended per head (so the PV matmul also
  produces the softmax denominator Z). Scores are computed transposed
  [sk,q] so the (mask, 1/8 scale, exp) all fuse into one ScalarE
  activation with a per-partition bias. xT[h] = v_aug^T @ exp gives the
  unnormalized context transposed plus the Z row; normalization
  multiplies by a PE-broadcast reciprocal of Z. The o-projection
  consumes xT directly as the stationary operand.
"""
import numpy as np

P = 128
B, S_ENC, S_DEC, D, H = 4, 2048, 1024, 1024, 16
HD = D // H                     # 64
NCORES = 8
FSH = 512                       # features per core (8 heads x 64)
HPC = 8                         # heads per core
NQT = S_DEC // P                # 8
NET = D // P                    # 8
NST = S_ENC // P                # 16
NFT = FSH // P                  # 4
VG = HD + 1                     # 65: v cols per head incl ones column

_NC = None


def _build_nc():
    from contextlib import ExitStack
    import concourse.bass as bass
    import concourse.tile as tile
    from concourse import bacc, mybir

    F32 = mybir.dt.float32
    F32R = mybir.dt.float32r
    ts = bass.ts
    Ident = mybir.ActivationFunctionType.Identity
    Exp = mybir.ActivationFunctionType.Exp

    nc = bacc.Bacc("TRN2", target_bir_lowering=False, debug=False)

    dec = nc.dram_tensor("dec", [S_DEC, D], F32R, kind="ExternalInput").ap()
    enc = nc.dram_tensor("enc", [S_ENC, D], F32R, kind="ExternalInput").ap()
    qw = nc.dram_tensor("qw", [D, FSH], F32R, kind="ExternalInput").ap()
    kw = nc.dram_tensor("kw", [D, FSH], F32R, kind="ExternalInput").ap()
    vw = nc.dram_tensor("vw", [D, FSH], F32R, kind="ExternalInput").ap()
    ow = nc.dram_tensor("ow", [FSH, D], F32R, kind="ExternalInput").ap()
    qb = nc.dram_tensor("qb", [P, NFT], F32, kind="ExternalInput").ap()
    kb = nc.dram_tensor("kb", [P, NFT], F32, kind="ExternalInput").ap()
    vb = nc.dram_tensor("vb", [1, FSH], F32R, kind="ExternalInput").ap()
    maskb = nc.dram_tensor("maskb", [P, NST], F32, kind="ExternalInput").ap()
    ident = nc.dram_tensor("ident", [P, P], F32R, kind="ExternalInput").ap()
    ones1 = nc.dram_tensor("ones1", [1, P], F32R, kind="ExternalInput").ap()
    onesb = nc.dram_tensor("onesb", [1, HD], F32R, kind="ExternalInput").ap()
    onescol = nc.dram_tensor("onescol", [P, HPC], F32R, kind="ExternalInput").ap()
    outp = nc.dram_tensor("outp", [S_DEC, D], F32, kind="ExternalOutput").ap()

    with tile.TileContext(nc) as tc, ExitStack() as ctx:
        const = ctx.enter_context(tc.tile_pool(name="const", bufs=1))
        ident_t = const.tile([P, P], F32R, tag="ident")
        ones1_t = const.tile([1, P], F32R, tag="ones1")
        onesb_t = const.tile([1, HD], F32R, tag="onesb")
        onescol_t = const.tile([P, HPC], F32R, tag="onescol")
        qb_t = const.tile([P, NFT], F32, tag="qb")
        kb_t = const.tile([P, NFT], F32, tag="kb")
        maskb_t = const.tile([P, NST], F32, tag="maskb")
        vb_t = const.tile([1, FSH], F32R, tag="vb")
        for t, src in ((ident_t, ident), (ones1_t, ones1), (onesb_t, onesb),
                       (onescol_t, onescol), (qb_t, qb), (kb_t, kb),
                       (maskb_t, maskb), (vb_t, vb)):
            nc.sync.dma_start(t[:], src[:])

        persist = ctx.enter_context(tc.tile_pool(name="persist", bufs=1))
        qT = [persist.tile([P, S_DEC], F32R, tag=f"qT{t}", name=f"qT{t}") for t in range(NFT)]
        kT = [persist.tile([P, S_ENC], F32R, tag=f"kT{t}", name=f"kT{t}") for t in range(NFT)]
        vt = [persist.tile([P, HPC * VG], F32R, tag=f"v{t}", name=f"v{t}") for t in range(NST)]
        xT = [persist.tile([P, S_DEC], F32R, tag=f"xT{t}", name=f"xT{t}") for t in range(NFT)]

        # ones column per head in the augmented v tiles
        for t in range(NST):
            dst = vt[t][:].rearrange("p (h c) -> p h c", h=HPC, c=VG)[:, :, HD:VG]
            nc.sync.dma_start(dst, onescol_t[:])

        # ---- stage 1+2: dec -> decT -> qT --------------------------------
        with tc.tile_pool(name="s12", bufs=1) as s12, \
             tc.tile_pool(name="decload", bufs=4) as decload, \
             tc.tile_pool(name="tps", bufs=6, space="PSUM") as tps, \
             tc.tile_pool(name="pqs", bufs=2, space="PSUM") as pqs:
            qw_t = [s12.tile([P, FSH], F32R, tag=f"qw{j}", name=f"qw{j}") for j in range(NET)]
            for j in range(NET):
                nc.sync.dma_start(qw_t[j][:], qw[ts(j, P), :])
            decT = [s12.tile([P, S_DEC], F32R, tag=f"decT{j}", name=f"decT{j}") for j in range(NET)]
            for g in range(2):
                dtiles = []
                for u in range(4):
                    i = g * 4 + u
                    dt_ = decload.tile([P, D], F32R, tag="decin")
                    nc.sync.dma_start(dt_[:], dec[ts(i, P), :])
                    dtiles.append(dt_)
                for j in range(NET):
                    pt = tps.tile([P, 512], F32R, tag="tp")
                    for u in range(4):
                        nc.tensor.transpose(pt[:, ts(u, P)],
                                            dtiles[u][:, ts(j, P)], ident_t[:])
                    nc.vector.tensor_copy(decT[j][:, ts(g, 512)], pt[:])
                for ft in range(NFT):
                    pq = pqs.tile([P, 512], F32, tag="pq")
                    for j in range(NET):
                        nc.tensor.matmul(pq[:], qw_t[j][:, ts(ft, P)],
                                         decT[j][:, ts(g, 512)],
                                         start=(j == 0), stop=(j == NET - 1))
                    nc.scalar.activation(qT[ft][:, ts(g, 512)], pq[:], Ident,
                                         bias=qb_t[:, ft:ft + 1])

        # ---- stage 3: enc -> encT -> kT, v (per sk-half) -----------------
        with tc.tile_pool(name="s3w", bufs=1) as s3w, \
             tc.tile_pool(name="encload", bufs=4) as encload, \
             tc.tile_pool(name="encTp", bufs=1) as encTp, \
             tc.tile_pool(name="tps3", bufs=6, space="PSUM") as tps3, \
             tc.tile_pool(name="pks", bufs=2, space="PSUM") as pks:
            kw_t = [s3w.tile([P, FSH], F32R, tag=f"kw{j}", name=f"kw{j}") for j in range(NET)]
            vw_t = [s3w.tile([P, FSH], F32R, tag=f"vw{j}", name=f"vw{j}") for j in range(NET)]
            for j in range(NET):
                nc.sync.dma_start(kw_t[j][:], kw[ts(j, P), :])
                nc.sync.dma_start(vw_t[j][:], vw[ts(j, P), :])
            for skh in range(2):
                encT = [encTp.tile([P, S_ENC // 2], F32R, tag=f"encT{j}", name=f"encT{j}")
                        for j in range(NET)]
                for g in range(2):
                    etiles = []
                    for u in range(4):
                        i = g * 4 + u
                        et = encload.tile([P, D], F32R, tag="encin")
                        nc.sync.dma_start(
                            et[:], enc[skh * (S_ENC // 2) + (g * 4 + u) * P:
                                       skh * (S_ENC // 2) + (g * 4 + u + 1) * P, :])
                        etiles.append(et)
                    for j in range(NET):
                        pt = tps3.tile([P, 512], F32R, tag="tp3")
                        for u in range(4):
                            nc.tensor.transpose(pt[:, ts(u, P)],
                                                etiles[u][:, ts(j, P)], ident_t[:])
                        nc.vector.tensor_copy(encT[j][:, ts(g, 512)], pt[:])
                for ft in range(NFT):
                    for sg in range(2):
                        pk = pks.tile([P, 512], F32, tag="pk")
                        for j in range(NET):
                            nc.tensor.matmul(pk[:], kw_t[j][:, ts(ft, P)],
                                             encT[j][:, ts(sg, 512)],
                                             start=(j == 0), stop=(j == NET - 1))
                        nc.scalar.activation(
                            kT[ft][:, skh * (S_ENC // 2) + sg * 512:
                                   skh * (S_ENC // 2) + (sg + 1) * 512],
                            pk[:], Ident, bias=kb_t[:, ft:ft + 1])
                for st in range(8):
                    pv = pks.tile([P, 512], F32, tag="pk")
                    for j in range(NET):
                        nc.tensor.matmul(pv[:], encT[j][:, ts(st, P)], vw_t[j][:],
                                         start=(j == 0), stop=False)
                    nc.tensor.matmul(pv[:], ones1_t[:], vb_t[:],
                                     start=False, stop=True)
                    dst = vt[skh * 8 + st][:].rearrange(
                        "p (h c) -> p h c", h=HPC, c=VG)[:, :, 0:HD]
                    nc.vector.tensor_copy(
                        dst, pv[:].rearrange("p (h c) -> p h c", h=HPC, c=HD))

        # ---- stage 4: attention per head + stage 5: o-projection ---------
        with tc.tile_pool(name="s5w", bufs=1) as s5w, \
             tc.tile_pool(name="outpool", bufs=3) as outpool:
            ow_t = [s5w.tile([P, D], F32R, tag=f"ow{t}", name=f"ow{t}")
                    for t in range(NFT)]
            for t in range(NFT):
                nc.sync.dma_start(ow_t[t][:], ow[ts(t, P), :])

            with tc.tile_pool(name="expp", bufs=3) as expp, \
                 tc.tile_pool(name="zp", bufs=2) as zp, \
                 tc.tile_pool(name="scps", bufs=2, space="PSUM") as scps, \
                 tc.tile_pool(name="xps", bufs=1, space="PSUM") as xps, \
                 tc.tile_pool(name="zps", bufs=1, space="PSUM") as zps:
                for h in range(HPC):
                    ft, r0 = h // 2, (h % 2) * HD
                    xp = xps.tile([VG, S_DEC], F32, tag="xp")
                    prev = None
                    for c in range(NST):
                        sc = scps.tile([P, S_DEC], F32, tag="sc")
                        for qh in range(2):
                            nc.tensor.matmul(sc[:, ts(qh, 512)],
                                             kT[ft][r0:r0 + HD, ts(c, P)],
                                             qT[ft][r0:r0 + HD, ts(qh, 512)],
                                             start=True, stop=True)
                        if prev is not None:
                            pc, pex = prev
                            for qh in range(2):
                                nc.tensor.matmul(xp[:, ts(qh, 512)],
                                                 vt[pc][:, pc_vslice(h)],
                                                 pex[:, ts(qh, 512)],
                                                 start=(pc == 0), stop=False)
                        ex = expp.tile([P, S_DEC], F32R, tag="ex")
                        nc.scalar.activation(ex[:], sc[:], Exp,
                                             bias=maskb_t[:, c:c + 1], scale=0.125)
                        prev = (c, ex)
                    pc, pex = prev
                    for qh in range(2):
                        nc.tensor.matmul(xp[:, ts(qh, 512)], vt[pc][:, pc_vslice(h)],
                                         pex[:, ts(qh, 512)],
                                         start=False, stop=True)
                    zrec = zp.tile([1, S_DEC], F32, tag="zrec")
                    nc.vector.reciprocal(zrec[:], xp[HD:VG, :])
                    zrecr = zp.tile([1, S_DEC], F32R, tag="zrecr")
                    nc.vector.tensor_copy(zrecr[:], zrec[:])
                    zb = zps.tile([HD, S_DEC], F32, tag="zb")
                    for qh in range(2):
                        nc.tensor.matmul(zb[:, ts(qh, 512)], onesb_t[:],
                                         zrecr[:, ts(qh, 512)], start=True, stop=True)
                    zbs = zp.tile([HD, S_DEC], F32, tag="zbs")
                    nc.vector.tensor_copy(zbs[:], zb[:])
                    nc.vector.tensor_mul(xT[ft][r0:r0 + HD, :], xp[0:HD, :], zbs[:])

            with tc.tile_pool(name="pops", bufs=2, space="PSUM") as pops:
                for qt in range(NQT):
                    ot = outpool.tile([P, D], F32, tag="ot")
                    for gh in range(2):
                        po = pops.tile([P, 512], F32, tag="po")
                        for fc in range(NFT):
                            nc.tensor.matmul(po[:], xT[fc][:, ts(qt, P)],
                                             ow_t[fc][:, ts(gh, 512)],
                                             start=(fc == 0), stop=(fc == NFT - 1))
                        nc.vector.tensor_copy(ot[:, ts(gh, 512)], po[:])
                    nc.sync.dma_start(outp[ts(qt, P), :], ot[:])

    nc.compile()
    return nc


def pc_vslice(h):
    import builtins
    return builtins.slice(h * VG, (h + 1) * VG)


def _get_nc():
    global _NC
    if _NC is None:
        _NC = _build_nc()
    return _NC


def make_in_maps(enc, enc_mask, dec, q_w, q_b, k_w, k_b, v_w, v_b, o_w, o_b):
    f32 = np.float32
    ca = np.ascontiguousarray
    in_maps = []
    for c in range(NCORES):
        b, hg = c // 2, c % 2
        fs = slice(hg * FSH, (hg + 1) * FSH)
        mb = np.where(np.asarray(enc_mask[b, 0, 0]), f32(-1e30), f32(0.0))
        in_maps.append({
            "dec": ca(np.asarray(dec[b], dtype=f32)),
            "enc": ca(np.asarray(enc[b], dtype=f32)),
            "qw": ca(np.asarray(q_w[:, fs], dtype=f32)),
            "kw": ca(np.asarray(k_w[:, fs], dtype=f32)),
            "vw": ca(np.asarray(v_w[:, fs], dtype=f32)),
            "ow": ca(np.asarray(o_w[fs, :], dtype=f32)),
            "qb": ca(np.asarray(q_b[fs], dtype=f32).reshape(NFT, P).T),
            "kb": ca(np.asarray(k_b[fs], dtype=f32).reshape(NFT, P).T),
            "vb": ca(np.asarray(v_b[fs], dtype=f32).reshape(1, FSH)),
            "maskb": ca(mb.astype(f32).reshape(NST, P).T),
            "ident": np.eye(P, dtype=f32),
            "ones1": np.ones((1, P), dtype=f32),
            "onesb": np.ones((1, HD), dtype=f32),
            "onescol": np.ones((P, HPC), dtype=f32),
        })
    return in_maps


def assemble(results, o_b):
    out = np.empty((B, S_DEC, D), dtype=np.float32)
    for b in range(B):
        out[b] = (results[2 * b]["outp"] + results[2 * b + 1]["outp"]
                  + np.asarray(o_b, dtype=np.float32))
    return out


def kernel(enc, enc_mask, dec, q_w, q_b, k_w, k_b, v_w, v_b, o_w, o_b):
    from concourse.bass_utils import run_bass_kernel_spmd
    nc = _get_nc()
    in_maps = make_in_maps(enc, enc_mask, dec, q_w, q_b, k_w, k_b,
                           v_w, v_b, o_w, o_b)
    res = run_bass_kernel_spmd(nc, in_maps, list(range(NCORES)))
    return assemble(res.results, o_b)
